# revision 20
# baseline (speedup 1.0000x reference)
"""LoraLinear (x @ W.T + 2*(x @ A.T) @ B.T) on 8 TRN2 NeuronCores.

Tensor-parallel: W and lora_B sharded row-wise (out_features) across the
8 cores; x and lora_A replicated. All transposition is done host-side so
each core streams its W.T shard with contiguous 1 MiB DMAs (the
memory-bound term: 32 MiB/core) while x.T tiles sit stationary in the PE.

Raw Bass (no Tile): this container's walrus rejects instructions carrying
more than a couple of attached sync-waits, so synchronization is explicit
standalone wait_ge instructions on a handful of semaphores.

Self-contained: shapes hardcoded for
  x [64, 4096] f32, weight [16384, 4096] f32,
  lora_A [64, 4096] f32, lora_B [16384, 64] f32  ->  out [64, 16384] f32
"""

import numpy as np

import concourse.bass as bass
import concourse.mybir as mybir
from concourse.bass_utils import run_bass_kernel_spmd

N_CORES = 8
TOK = 64          # tokens
IN_F = 4096       # in_features (contraction)
OUT_F = 16384     # out_features
R = 64            # lora rank
SCALING = 2.0
O_SHARD = OUT_F // N_CORES   # 2048 out features per core
P = 128
KT = IN_F // P               # 32 k-tiles
NB = O_SHARD // 512          # 4 psum blocks of 512
NBUF = 8                     # W slab prefetch ring depth
F32 = mybir.dt.float32

# float32r: fp32 bits on the PE fast path — 1 cycle/row at moving>=256 vs
# 4 for plain fp32. The walrus BIR verifier requires f32r matmul operands
# to be produced in f32r dtype, so the whole W/x/A/B path (DRAM + SBUF) is
# declared float32r (same 4-byte fp32 bits on the host). Slightly reduced
# product mantissa; harness tolerance is 2e-2, orders of magnitude away.
USE_F32R = True
USE_DMA2 = True              # W fetched as 16 x 2 MiB DMAs (slab pairs)


def _build_nc(reps=1, w_only=False):
    """reps>1 loops the whole computation inside one NEFF (bench only):
    per-iteration semaphore targets are offset so the pipeline stays
    correct across iterations. w_only=True loads xt/at/bt once and
    re-streams only W per iteration (bandwidth calibration)."""
    nc = bass.Bass()
    # Host-prepared layouts (see _prep_in_maps):
    #   xt  [128, KT*64]  x.T in SBUF partition-major k-tile layout
    #   at  [128, KT*64]  (SCALING*lora_A).T in the same layout
    #   wt  [4096, 2048]  per-core W shard, transposed (k rows, o cols)
    #   bt  [64, 2048]    per-core lora_B shard, transposed (r rows, o cols)
    FIN = mybir.dt.float32r if USE_F32R else F32
    xt = nc.dram_tensor("xt", [P, KT * TOK], FIN, kind="ExternalInput")
    at = nc.dram_tensor("at", [P, KT * TOK], FIN, kind="ExternalInput")
    wt = nc.dram_tensor("wt", [IN_F, O_SHARD], FIN, kind="ExternalInput")
    bt = nc.dram_tensor("bt", [R, O_SHARD], FIN, kind="ExternalInput")
    out = nc.dram_tensor("out", [TOK, O_SHARD], F32, kind="ExternalOutput")

    with (
        nc.sbuf_tensor("xt_sb", [P, KT, TOK], FIN) as xt_sb,
        nc.sbuf_tensor("at_sb", [P, KT, TOK], FIN) as at_sb,
        nc.sbuf_tensor("bt_sb", [R, O_SHARD], FIN) as bt_sb,
        nc.sbuf_tensor("ut_sb", [R, TOK], FIN) as ut_sb,
        nc.sbuf_tensor("w_sb", [P, NBUF, O_SHARD], FIN) as w_sb,
        nc.sbuf_tensor("out_sb", [TOK, O_SHARD], F32) as out_sb,
        nc.psum_tensor("ps_o", [TOK, NB, 512], F32) as ps_o,
        nc.psum_tensor("ps_ut", [R, TOK], F32) as ps_ut,
        nc.semaphore("in_sem") as in_sem,     # xt/at/bt DMA done (+16 each)
        nc.semaphore("w_sem") as w_sem,       # W slab DMA done (+16 each)
        nc.semaphore("slot_sem") as slot_sem, # PE done with slab k (+1)
        nc.semaphore("pe_sem") as pe_sem,     # PE milestones (+1)
        nc.semaphore("cp_sem") as cp_sem,     # DVE copies done (+1)
        nc.semaphore("done_sem") as done_sem, # out DMA done (+16)
        nc.Block() as block,
    ):

        @block.sync
        def _(sync):
            # Input stream only: xt/at/bt then the 32 W slabs, all reps
            # back-to-back. Output DMAs live on the gpsimd queue so the W
            # stream never stalls on compute completion.
            for it in range(reps):
                if it == 0 or not w_only:
                    if it > 0:
                        # prior iteration's PE must be done with xt/at/bt:
                        # its slab-31 (last-issued) matmul released slot_sem
                        sync.wait_ge(slot_sem, it * KT)
                    sync.dma_start(
                        out=xt_sb[:], in_=xt.rearrange("p (kt t) -> p kt t", kt=KT)
                    ).then_inc(in_sem, 16)
                    sync.dma_start(
                        out=at_sb[:], in_=at.rearrange("p (kt t) -> p kt t", kt=KT)
                    ).then_inc(in_sem, 16)
                    sync.dma_start(out=bt_sb[:], in_=bt[:]).then_inc(in_sem, 16)
                if USE_DMA2:
                    # slab pairs: one 2 MiB DMA fills two adjacent ring
                    # slots; partition p takes rows {2j*128+p, 2j*128+128+p}
                    for j in range(KT // 2):
                        gk = it * KT + 2 * j
                        if gk + 1 >= NBUF:
                            sync.wait_ge(slot_sem, gk + 2 - NBUF)
                        s = gk % NBUF
                        sync.dma_start(
                            out=w_sb[:, s:s + 2, :],
                            in_=wt[2 * j * P:(2 * j + 2) * P, :].rearrange(
                                "(two p) o -> p two o", p=P),
                        ).then_inc(w_sem, 16)
                else:
                    for k in range(KT):
                        gk = it * KT + k
                        if gk >= NBUF:
                            sync.wait_ge(slot_sem, gk - NBUF + 1)
                        sync.dma_start(
                            out=w_sb[:, gk % NBUF, :], in_=wt[k * P:(k + 1) * P, :]
                        ).then_inc(w_sem, 16)
            sync.wait_ge(done_sem, 16 * NB * reps)

        @block.gpsimd
        def _(gpsimd):
            # Per-bank output store: bank b leaves as soon as its copyback
            # lands, overlapping the tail of the W stream.
            for it in range(reps):
                base_cp = it * (NB + 1)
                for b in range(NB):
                    gpsimd.wait_ge(cp_sem, base_cp + 2 + b)
                    gpsimd.dma_start(
                        out=out[:, b * 512:(b + 1) * 512],
                        in_=out_sb[:, b * 512:(b + 1) * 512],
                    ).then_inc(done_sem, 16)

        @block.tensor
        def _(tensor):
            # pe_sem per iter: +1 ut accumulation, +1 per stop-matmul of
            # banks 0..NB-2 (bank NB-1's stop doubles as the slab-31 slot
            # release on slot_sem, which the last copyback keys on).
            for it in range(reps):
                base_in = 0 if w_only else it * 48
                base_pe = it * NB
                base_cp = it * (NB + 1)
                # lora prologue (hidden under the W DMA stream): uT =
                # (SCALING*A) @ x.T, then psum[t, o] = uT.T @ bT with
                # start=True so the k-loop's last matmul can carry stop.
                tensor.wait_ge(in_sem, base_in + 32)   # xt + at resident
                for j in range(KT):
                    mmu = nc.tensor.matmul(
                        ps_ut[:], at_sb[:, j, :], xt_sb[:, j, :],
                        start=(j == 0), stop=(j == KT - 1))
                mmu.then_inc(pe_sem, 1)
                tensor.wait_ge(in_sem, base_in + 48)   # bt resident
                tensor.wait_ge(cp_sem, base_cp + 1)    # ut_sb written by DVE
                for b in range(NB):
                    if it > 0:
                        # prior iteration's bank-b copyback must finish
                        tensor.wait_ge(cp_sem, (it - 1) * (NB + 1) + 2 + b)
                    nc.tensor.matmul(
                        ps_o[:, b, :], ut_sb[:],
                        bt_sb[:, b * 512:(b + 1) * 512],
                        start=True, stop=False)
                for k in range(KT):
                    gk = it * KT + k
                    if USE_DMA2:
                        if k % 2 == 0:
                            tensor.wait_ge(w_sem, 16 * (gk // 2 + 1))
                    else:
                        tensor.wait_ge(w_sem, 16 * (gk + 1))
                    for b in range(NB):
                        mm = nc.tensor.matmul(
                            ps_o[:, b, :], xt_sb[:, k, :],
                            w_sb[:, gk % NBUF, b * 512:(b + 1) * 512],
                            start=False, stop=(k == KT - 1))
                        if k == KT - 1 and b < NB - 1:
                            mm.then_inc(pe_sem, 1)
                        elif b == NB - 1:
                            mm.then_inc(slot_sem, 1)

        @block.vector
        def _(vector):
            for it in range(reps):
                base_pe = it * NB
                vector.wait_ge(pe_sem, base_pe + 1)    # ut accumulation done
                nc.vector.tensor_copy(out=ut_sb[:], in_=ps_ut[:]).then_inc(cp_sem, 1)
                for b in range(NB):
                    if b < NB - 1:
                        vector.wait_ge(pe_sem, base_pe + 2 + b)  # bank stopped
                    else:
                        vector.wait_ge(slot_sem, (it + 1) * KT)  # slab31 done
                    if it > 0:
                        # bank b's prior-iteration store must have left SBUF
                        vector.wait_ge(done_sem, (it - 1) * 16 * NB + 16 * (b + 1))
                    nc.vector.tensor_copy(
                        out=out_sb[:, b * 512:(b + 1) * 512], in_=ps_o[:, b, :]
                    ).then_inc(cp_sem, 1)

    return nc


_NC_CACHE = {}


def _get_nc(reps=1, w_only=False):
    key = (reps, w_only)
    if key not in _NC_CACHE:
        _NC_CACHE[key] = _build_nc(reps, w_only)
    return _NC_CACHE[key]


def _prep_in_maps(x, weight, lora_A, lora_B):
    # x.T in SBUF partition-major layout: [4096,64] -> [KT,128,64] -> [128, KT*64]
    xt = np.ascontiguousarray(
        x.T.reshape(KT, P, TOK).transpose(1, 0, 2).reshape(P, KT * TOK))
    at = np.ascontiguousarray(
        (SCALING * lora_A).T.reshape(KT, P, TOK).transpose(1, 0, 2).reshape(P, KT * TOK))
    wt_full = np.ascontiguousarray(weight.T)          # [4096, 16384]
    bt_full = np.ascontiguousarray(lora_B.T)          # [64, 16384]
    in_maps = []
    for c in range(N_CORES):
        sl = slice(c * O_SHARD, (c + 1) * O_SHARD)
        in_maps.append({
            "xt": xt,
            "at": at,
            "wt": np.ascontiguousarray(wt_full[:, sl]),
            "bt": np.ascontiguousarray(bt_full[:, sl]),
        })
    return in_maps


def kernel(x, weight, lora_A, lora_B, trace=False):
    x = np.asarray(x, dtype=np.float32)
    weight = np.asarray(weight, dtype=np.float32)
    lora_A = np.asarray(lora_A, dtype=np.float32)
    lora_B = np.asarray(lora_B, dtype=np.float32)
    nc = _get_nc()
    in_maps = _prep_in_maps(x, weight, lora_A, lora_B)
    res = run_bass_kernel_spmd(nc, in_maps, core_ids=list(range(N_CORES)),
                               trace=trace)
    out = np.concatenate([res.results[c]["out"] for c in range(N_CORES)], axis=1)
    if trace:
        kernel.last_results = res
    return out


# revision 21
# speedup vs baseline: 1.0623x; 1.0623x over previous
"""LoraLinear (x @ W.T + 2*(x @ A.T) @ B.T) on 8 TRN2 NeuronCores.

Tensor-parallel: W and lora_B sharded row-wise (out_features) across the
8 cores; x and lora_A replicated. All transposition is done host-side so
each core streams its W.T shard with contiguous 1 MiB DMAs (the
memory-bound term: 32 MiB/core) while x.T tiles sit stationary in the PE.

Raw Bass (no Tile): this container's walrus rejects instructions carrying
more than a couple of attached sync-waits, so synchronization is explicit
standalone wait_ge instructions on a handful of semaphores.

Self-contained: shapes hardcoded for
  x [64, 4096] f32, weight [16384, 4096] f32,
  lora_A [64, 4096] f32, lora_B [16384, 64] f32  ->  out [64, 16384] f32
"""

import numpy as np

import concourse.bass as bass
import concourse.mybir as mybir
from concourse.bass_utils import run_bass_kernel_spmd

N_CORES = 8
TOK = 64          # tokens
IN_F = 4096       # in_features (contraction)
OUT_F = 16384     # out_features
R = 64            # lora rank
SCALING = 2.0
O_SHARD = OUT_F // N_CORES   # 2048 out features per core
P = 128
KT = IN_F // P               # 32 k-tiles
NB = O_SHARD // 512          # 4 psum blocks of 512
NBUF = 8                     # W slab prefetch ring depth
F32 = mybir.dt.float32

# float32r: fp32 bits on the PE fast path — 1 cycle/row at moving>=256 vs
# 4 for plain fp32. The walrus BIR verifier requires f32r matmul operands
# to be produced in f32r dtype, so the whole W/x/A/B path (DRAM + SBUF) is
# declared float32r (same 4-byte fp32 bits on the host). Slightly reduced
# product mantissa; harness tolerance is 2e-2, orders of magnitude away.
USE_F32R = True
USE_DMA2 = False             # W fetched as 16 x 2 MiB DMAs (slab pairs)


def _build_nc(reps=1, w_only=False):
    """reps>1 loops the whole computation inside one NEFF (bench only):
    per-iteration semaphore targets are offset so the pipeline stays
    correct across iterations. w_only=True loads xt/at/bt once and
    re-streams only W per iteration (bandwidth calibration)."""
    nc = bass.Bass()
    # Host-prepared layouts (see _prep_in_maps):
    #   xt  [128, KT*64]  x.T in SBUF partition-major k-tile layout
    #   at  [128, KT*64]  (SCALING*lora_A).T in the same layout
    #   wt  [4096, 2048]  per-core W shard, transposed (k rows, o cols)
    #   bt  [64, 2048]    per-core lora_B shard, transposed (r rows, o cols)
    FIN = mybir.dt.float32r if USE_F32R else F32
    xt = nc.dram_tensor("xt", [P, KT * TOK], FIN, kind="ExternalInput")
    at = nc.dram_tensor("at", [P, KT * TOK], FIN, kind="ExternalInput")
    wt = nc.dram_tensor("wt", [IN_F, O_SHARD], FIN, kind="ExternalInput")
    bt = nc.dram_tensor("bt", [R, O_SHARD], FIN, kind="ExternalInput")
    out = nc.dram_tensor("out", [TOK, O_SHARD], F32, kind="ExternalOutput")

    with (
        nc.sbuf_tensor("xt_sb", [P, KT, TOK], FIN) as xt_sb,
        nc.sbuf_tensor("at_sb", [P, KT, TOK], FIN) as at_sb,
        nc.sbuf_tensor("bt_sb", [R, O_SHARD], FIN) as bt_sb,
        nc.sbuf_tensor("ut_sb", [R, TOK], FIN) as ut_sb,
        nc.sbuf_tensor("w_sb", [P, NBUF, O_SHARD], FIN) as w_sb,
        nc.sbuf_tensor("out_sb", [TOK, O_SHARD], F32) as out_sb,
        nc.psum_tensor("ps_o", [TOK, NB, 512], F32) as ps_o,
        nc.psum_tensor("ps_ut", [R, TOK], F32) as ps_ut,
        nc.semaphore("in_sem") as in_sem,     # xt/at/bt DMA done (+16 each)
        nc.semaphore("w_sem") as w_sem,       # W slab DMA done (+16 each)
        nc.semaphore("slot_sem") as slot_sem, # PE done with slab k (+1)
        nc.semaphore("pe_sem") as pe_sem,     # PE milestones (+1)
        nc.semaphore("cp_sem") as cp_sem,     # DVE copies done (+1)
        nc.semaphore("done_sem") as done_sem, # out DMA done (+16)
        nc.Block() as block,
    ):

        @block.sync
        def _(sync):
            # Input stream only: xt/at/bt then the 32 W slabs, all reps
            # back-to-back. Output DMAs live on the gpsimd queue so the W
            # stream never stalls on compute completion.
            for it in range(reps):
                if it == 0 or not w_only:
                    if it > 0:
                        # prior iteration's PE must be done with xt/at/bt:
                        # its slab-31 (last-issued) matmul released slot_sem
                        sync.wait_ge(slot_sem, it * KT)
                    sync.dma_start(
                        out=xt_sb[:], in_=xt.rearrange("p (kt t) -> p kt t", kt=KT)
                    ).then_inc(in_sem, 16)
                    sync.dma_start(
                        out=at_sb[:], in_=at.rearrange("p (kt t) -> p kt t", kt=KT)
                    ).then_inc(in_sem, 16)
                    sync.dma_start(out=bt_sb[:], in_=bt[:]).then_inc(in_sem, 16)
                if USE_DMA2:
                    # slab pairs: one 2 MiB DMA fills two adjacent ring
                    # slots; partition p takes rows {2j*128+p, 2j*128+128+p}
                    for j in range(KT // 2):
                        gk = it * KT + 2 * j
                        if gk + 1 >= NBUF:
                            sync.wait_ge(slot_sem, gk + 2 - NBUF)
                        s = gk % NBUF
                        sync.dma_start(
                            out=w_sb[:, s:s + 2, :],
                            in_=wt[2 * j * P:(2 * j + 2) * P, :].rearrange(
                                "(two p) o -> p two o", p=P),
                        ).then_inc(w_sem, 16)
                else:
                    for k in range(KT):
                        gk = it * KT + k
                        if gk >= NBUF:
                            sync.wait_ge(slot_sem, gk - NBUF + 1)
                        sync.dma_start(
                            out=w_sb[:, gk % NBUF, :], in_=wt[k * P:(k + 1) * P, :]
                        ).then_inc(w_sem, 16)
            sync.wait_ge(done_sem, 16 * NB * reps)

        @block.gpsimd
        def _(gpsimd):
            # Per-bank output store: bank b leaves as soon as its copyback
            # lands, overlapping the tail of the W stream.
            for it in range(reps):
                base_cp = it * (NB + 1)
                for b in range(NB):
                    gpsimd.wait_ge(cp_sem, base_cp + 2 + b)
                    gpsimd.dma_start(
                        out=out[:, b * 512:(b + 1) * 512],
                        in_=out_sb[:, b * 512:(b + 1) * 512],
                    ).then_inc(done_sem, 16)

        @block.tensor
        def _(tensor):
            # pe_sem per iter: +1 ut accumulation, +1 per stop-matmul of
            # banks 0..NB-2 (bank NB-1's stop doubles as the slab-31 slot
            # release on slot_sem, which the last copyback keys on).
            for it in range(reps):
                base_in = 0 if w_only else it * 48
                base_pe = it * NB
                base_cp = it * (NB + 1)
                # lora prologue (hidden under the W DMA stream): uT =
                # (SCALING*A) @ x.T, then psum[t, o] = uT.T @ bT with
                # start=True so the k-loop's last matmul can carry stop.
                tensor.wait_ge(in_sem, base_in + 32)   # xt + at resident
                for j in range(KT):
                    mmu = nc.tensor.matmul(
                        ps_ut[:], at_sb[:, j, :], xt_sb[:, j, :],
                        start=(j == 0), stop=(j == KT - 1))
                mmu.then_inc(pe_sem, 1)
                tensor.wait_ge(in_sem, base_in + 48)   # bt resident
                tensor.wait_ge(cp_sem, base_cp + 1)    # ut_sb written by DVE
                for b in range(NB):
                    if it > 0:
                        # prior iteration's bank-b copyback must finish
                        tensor.wait_ge(cp_sem, (it - 1) * (NB + 1) + 2 + b)
                    nc.tensor.matmul(
                        ps_o[:, b, :], ut_sb[:],
                        bt_sb[:, b * 512:(b + 1) * 512],
                        start=True, stop=False)
                for k in range(KT):
                    gk = it * KT + k
                    if USE_DMA2:
                        if k % 2 == 0:
                            tensor.wait_ge(w_sem, 16 * (gk // 2 + 1))
                    else:
                        tensor.wait_ge(w_sem, 16 * (gk + 1))
                    for b in range(NB):
                        mm = nc.tensor.matmul(
                            ps_o[:, b, :], xt_sb[:, k, :],
                            w_sb[:, gk % NBUF, b * 512:(b + 1) * 512],
                            start=False, stop=(k == KT - 1))
                        if k == KT - 1 and b < NB - 1:
                            mm.then_inc(pe_sem, 1)
                        elif b == NB - 1:
                            mm.then_inc(slot_sem, 1)

        @block.vector
        def _(vector):
            for it in range(reps):
                base_pe = it * NB
                vector.wait_ge(pe_sem, base_pe + 1)    # ut accumulation done
                nc.vector.tensor_copy(out=ut_sb[:], in_=ps_ut[:]).then_inc(cp_sem, 1)
                for b in range(NB):
                    if b < NB - 1:
                        vector.wait_ge(pe_sem, base_pe + 2 + b)  # bank stopped
                    else:
                        vector.wait_ge(slot_sem, (it + 1) * KT)  # slab31 done
                    if it > 0:
                        # bank b's prior-iteration store must have left SBUF
                        vector.wait_ge(done_sem, (it - 1) * 16 * NB + 16 * (b + 1))
                    nc.vector.tensor_copy(
                        out=out_sb[:, b * 512:(b + 1) * 512], in_=ps_o[:, b, :]
                    ).then_inc(cp_sem, 1)

    return nc


_NC_CACHE = {}


def _get_nc(reps=1, w_only=False):
    key = (reps, w_only)
    if key not in _NC_CACHE:
        _NC_CACHE[key] = _build_nc(reps, w_only)
    return _NC_CACHE[key]


def _prep_in_maps(x, weight, lora_A, lora_B):
    # x.T in SBUF partition-major layout: [4096,64] -> [KT,128,64] -> [128, KT*64]
    xt = np.ascontiguousarray(
        x.T.reshape(KT, P, TOK).transpose(1, 0, 2).reshape(P, KT * TOK))
    at = np.ascontiguousarray(
        (SCALING * lora_A).T.reshape(KT, P, TOK).transpose(1, 0, 2).reshape(P, KT * TOK))
    wt_full = np.ascontiguousarray(weight.T)          # [4096, 16384]
    bt_full = np.ascontiguousarray(lora_B.T)          # [64, 16384]
    in_maps = []
    for c in range(N_CORES):
        sl = slice(c * O_SHARD, (c + 1) * O_SHARD)
        in_maps.append({
            "xt": xt,
            "at": at,
            "wt": np.ascontiguousarray(wt_full[:, sl]),
            "bt": np.ascontiguousarray(bt_full[:, sl]),
        })
    return in_maps


def kernel(x, weight, lora_A, lora_B, trace=False):
    x = np.asarray(x, dtype=np.float32)
    weight = np.asarray(weight, dtype=np.float32)
    lora_A = np.asarray(lora_A, dtype=np.float32)
    lora_B = np.asarray(lora_B, dtype=np.float32)
    nc = _get_nc()
    in_maps = _prep_in_maps(x, weight, lora_A, lora_B)
    res = run_bass_kernel_spmd(nc, in_maps, core_ids=list(range(N_CORES)),
                               trace=trace)
    out = np.concatenate([res.results[c]["out"] for c in range(N_CORES)], axis=1)
    if trace:
        kernel.last_results = res
    return out


# revision 22
# speedup vs baseline: 1.1662x; 1.0978x over previous
"""LoraLinear (x @ W.T + 2*(x @ A.T) @ B.T) on 8 TRN2 NeuronCores.

Tensor-parallel: W and lora_B sharded row-wise (out_features) across the
8 cores; x and lora_A replicated. All transposition is done host-side so
each core streams its W.T shard with contiguous 1 MiB DMAs (the
memory-bound term: 32 MiB/core) while x.T tiles sit stationary in the PE.

Raw Bass (no Tile): this container's walrus rejects instructions carrying
more than a couple of attached sync-waits, so synchronization is explicit
standalone wait_ge instructions on a handful of semaphores.

Self-contained: shapes hardcoded for
  x [64, 4096] f32, weight [16384, 4096] f32,
  lora_A [64, 4096] f32, lora_B [16384, 64] f32  ->  out [64, 16384] f32
"""

import numpy as np

import concourse.bass as bass
import concourse.mybir as mybir
from concourse.bass_utils import run_bass_kernel_spmd

N_CORES = 8
TOK = 64          # tokens
IN_F = 4096       # in_features (contraction)
OUT_F = 16384     # out_features
R = 64            # lora rank
SCALING = 2.0
O_SHARD = OUT_F // N_CORES   # 2048 out features per core
P = 128
KT = IN_F // P               # 32 k-tiles
NB = O_SHARD // 512          # 4 psum blocks of 512
NBUF = 8                     # W slab prefetch ring depth
F32 = mybir.dt.float32

# float32r: fp32 bits on the PE fast path — 1 cycle/row at moving>=256 vs
# 4 for plain fp32. The walrus BIR verifier requires f32r matmul operands
# to be produced in f32r dtype, so the whole W/x/A/B path (DRAM + SBUF) is
# declared float32r (same 4-byte fp32 bits on the host). Slightly reduced
# product mantissa; harness tolerance is 2e-2, orders of magnitude away.
USE_F32R = True
USE_DMA2 = False             # W fetched as 16 x 2 MiB DMAs (slab pairs)


def _build_nc(reps=1, w_only=False):
    """reps>1 loops the whole computation inside one NEFF (bench only):
    per-iteration semaphore targets are offset so the pipeline stays
    correct across iterations. w_only=True loads xt/at/bt once and
    re-streams only W per iteration (bandwidth calibration)."""
    nc = bass.Bass()
    # Host-prepared layouts (see _prep_in_maps):
    #   xt  [128, KT*64]  x.T in SBUF partition-major k-tile layout
    #   at  [128, KT*64]  (SCALING*lora_A).T in the same layout
    #   wt  [4096, 2048]  per-core W shard, transposed (k rows, o cols)
    #   bt  [64, 2048]    per-core lora_B shard, transposed (r rows, o cols)
    FIN = mybir.dt.float32r if USE_F32R else F32
    xt = nc.dram_tensor("xt", [P, KT * TOK], FIN, kind="ExternalInput")
    at = nc.dram_tensor("at", [P, KT * TOK], FIN, kind="ExternalInput")
    wt = nc.dram_tensor("wt", [IN_F, O_SHARD], FIN, kind="ExternalInput")
    bt = nc.dram_tensor("bt", [R, O_SHARD], FIN, kind="ExternalInput")
    out = nc.dram_tensor("out", [TOK, O_SHARD], F32, kind="ExternalOutput")

    with (
        nc.sbuf_tensor("xt_sb", [P, KT, TOK], FIN) as xt_sb,
        nc.sbuf_tensor("at_sb", [P, KT, TOK], FIN) as at_sb,
        nc.sbuf_tensor("bt_sb", [R, O_SHARD], FIN) as bt_sb,
        nc.sbuf_tensor("ut_sb", [R, TOK], FIN) as ut_sb,
        nc.sbuf_tensor("w_sb", [P, NBUF, O_SHARD], FIN) as w_sb,
        nc.sbuf_tensor("out_sb", [TOK, O_SHARD], F32) as out_sb,
        nc.psum_tensor("ps_o", [TOK, NB, 512], F32) as ps_o,
        nc.psum_tensor("ps_ut", [R, TOK], F32) as ps_ut,
        nc.semaphore("in_sem") as in_sem,     # xt/at DMA done (+16 each)
        nc.semaphore("bt_sem") as bt_sem,     # bt DMA done (+16)
        nc.semaphore("w_sem") as w_sem,       # W slab DMA done (+16 each)
        nc.semaphore("slot_sem") as slot_sem, # PE done with slab k (+1)
        nc.semaphore("pe_sem") as pe_sem,     # PE milestones (+1)
        nc.semaphore("cp_sem") as cp_sem,     # DVE copies done (+1)
        nc.semaphore("done_sem") as done_sem, # out DMA done (+16)
        nc.Block() as block,
    ):

        @block.sync
        def _(sync):
            # Input stream only: xt/at/bt then the 32 W slabs, all reps
            # back-to-back. Output DMAs live on the gpsimd queue so the W
            # stream never stalls on compute completion.
            for it in range(reps):
                if it == 0 or not w_only:
                    if it > 0:
                        # prior iteration's PE must be done with xt/at/bt:
                        # its slab-31 (last-issued) matmul released slot_sem
                        sync.wait_ge(slot_sem, it * KT)
                    sync.dma_start(
                        out=xt_sb[:], in_=xt.rearrange("p (kt t) -> p kt t", kt=KT)
                    ).then_inc(in_sem, 16)
                    sync.dma_start(
                        out=at_sb[:], in_=at.rearrange("p (kt t) -> p kt t", kt=KT)
                    ).then_inc(in_sem, 16)
                    sync.dma_start(out=bt_sb[:], in_=bt[:]).then_inc(bt_sem, 16)
                if USE_DMA2:
                    # slab pairs: one 2 MiB DMA fills two adjacent ring
                    # slots; partition p takes rows {2j*128+p, 2j*128+128+p}
                    for j in range(KT // 2):
                        gk = it * KT + 2 * j
                        if gk + 1 >= NBUF:
                            sync.wait_ge(slot_sem, gk + 2 - NBUF)
                        s = gk % NBUF
                        sync.dma_start(
                            out=w_sb[:, s:s + 2, :],
                            in_=wt[2 * j * P:(2 * j + 2) * P, :].rearrange(
                                "(two p) o -> p two o", p=P),
                        ).then_inc(w_sem, 16)
                else:
                    for k in range(KT):
                        gk = it * KT + k
                        if gk >= NBUF:
                            sync.wait_ge(slot_sem, gk - NBUF + 1)
                        sync.dma_start(
                            out=w_sb[:, gk % NBUF, :], in_=wt[k * P:(k + 1) * P, :]
                        ).then_inc(w_sem, 16)
            sync.wait_ge(done_sem, 16 * NB * reps)

        @block.gpsimd
        def _(gpsimd):
            # Per-bank output store: bank b leaves as soon as its copyback
            # lands, overlapping the tail of the W stream.
            for it in range(reps):
                base_cp = it * (NB + 1)
                for b in range(NB):
                    gpsimd.wait_ge(cp_sem, base_cp + 2 + b)
                    gpsimd.dma_start(
                        out=out[:, b * 512:(b + 1) * 512],
                        in_=out_sb[:, b * 512:(b + 1) * 512],
                    ).then_inc(done_sem, 16)

        @block.tensor
        def _(tensor):
            # pe_sem per iter: +1 ut accumulation, +1 per stop-matmul of
            # banks 0..NB-2 (bank NB-1's stop doubles as the slab-31 slot
            # release on slot_sem, which the last copyback keys on).
            for it in range(reps):
                base_in = 0 if w_only else it * 32
                base_bt = 0 if w_only else it * 16
                base_pe = it * NB
                base_cp = it * (NB + 1)
                # lora prologue (hidden under the W DMA stream): uT =
                # (SCALING*A) @ x.T, then psum[t, o] = uT.T @ bT with
                # start=True so the k-loop's last matmul can carry stop.
                tensor.wait_ge(in_sem, base_in + 32)   # xt + at resident
                for j in range(KT):
                    mmu = nc.tensor.matmul(
                        ps_ut[:], at_sb[:, j, :], xt_sb[:, j, :],
                        start=(j == 0), stop=(j == KT - 1))
                mmu.then_inc(pe_sem, 1)
                tensor.wait_ge(bt_sem, base_bt + 16)   # bt resident
                tensor.wait_ge(cp_sem, base_cp + 1)    # ut_sb written by DVE
                for b in range(NB):
                    if it > 0:
                        # prior iteration's bank-b copyback must finish
                        tensor.wait_ge(cp_sem, (it - 1) * (NB + 1) + 2 + b)
                    nc.tensor.matmul(
                        ps_o[:, b, :], ut_sb[:],
                        bt_sb[:, b * 512:(b + 1) * 512],
                        start=True, stop=False)
                for k in range(KT):
                    gk = it * KT + k
                    if USE_DMA2:
                        if k % 2 == 0:
                            tensor.wait_ge(w_sem, 16 * (gk // 2 + 1))
                    else:
                        tensor.wait_ge(w_sem, 16 * (gk + 1))
                    for b in range(NB):
                        mm = nc.tensor.matmul(
                            ps_o[:, b, :], xt_sb[:, k, :],
                            w_sb[:, gk % NBUF, b * 512:(b + 1) * 512],
                            start=False, stop=(k == KT - 1))
                        if k == KT - 1 and b < NB - 1:
                            mm.then_inc(pe_sem, 1)
                        elif b == NB - 1:
                            mm.then_inc(slot_sem, 1)

        @block.vector
        def _(vector):
            for it in range(reps):
                base_pe = it * NB
                vector.wait_ge(pe_sem, base_pe + 1)    # ut accumulation done
                nc.vector.tensor_copy(out=ut_sb[:], in_=ps_ut[:]).then_inc(cp_sem, 1)
                for b in range(NB):
                    if b < NB - 1:
                        vector.wait_ge(pe_sem, base_pe + 2 + b)  # bank stopped
                    else:
                        vector.wait_ge(slot_sem, (it + 1) * KT)  # slab31 done
                    if it > 0:
                        # bank b's prior-iteration store must have left SBUF
                        vector.wait_ge(done_sem, (it - 1) * 16 * NB + 16 * (b + 1))
                    nc.vector.tensor_copy(
                        out=out_sb[:, b * 512:(b + 1) * 512], in_=ps_o[:, b, :]
                    ).then_inc(cp_sem, 1)

    return nc


_NC_CACHE = {}


def _get_nc(reps=1, w_only=False):
    key = (reps, w_only)
    if key not in _NC_CACHE:
        _NC_CACHE[key] = _build_nc(reps, w_only)
    return _NC_CACHE[key]


def _prep_in_maps(x, weight, lora_A, lora_B):
    # x.T in SBUF partition-major layout: [4096,64] -> [KT,128,64] -> [128, KT*64]
    xt = np.ascontiguousarray(
        x.T.reshape(KT, P, TOK).transpose(1, 0, 2).reshape(P, KT * TOK))
    at = np.ascontiguousarray(
        (SCALING * lora_A).T.reshape(KT, P, TOK).transpose(1, 0, 2).reshape(P, KT * TOK))
    wt_full = np.ascontiguousarray(weight.T)          # [4096, 16384]
    bt_full = np.ascontiguousarray(lora_B.T)          # [64, 16384]
    in_maps = []
    for c in range(N_CORES):
        sl = slice(c * O_SHARD, (c + 1) * O_SHARD)
        in_maps.append({
            "xt": xt,
            "at": at,
            "wt": np.ascontiguousarray(wt_full[:, sl]),
            "bt": np.ascontiguousarray(bt_full[:, sl]),
        })
    return in_maps


def kernel(x, weight, lora_A, lora_B, trace=False):
    x = np.asarray(x, dtype=np.float32)
    weight = np.asarray(weight, dtype=np.float32)
    lora_A = np.asarray(lora_A, dtype=np.float32)
    lora_B = np.asarray(lora_B, dtype=np.float32)
    nc = _get_nc()
    in_maps = _prep_in_maps(x, weight, lora_A, lora_B)
    res = run_bass_kernel_spmd(nc, in_maps, core_ids=list(range(N_CORES)),
                               trace=trace)
    out = np.concatenate([res.results[c]["out"] for c in range(N_CORES)], axis=1)
    if trace:
        kernel.last_results = res
    return out
